# revision 12
# baseline (speedup 1.0000x reference)
"""BoundaryChunker Trainium2 kernel.

Strategy (data-parallel over batch: row r -> NeuronCore r):
  host:   per-row boundary positions, window starts s0 = max(0, t-4), combine
          coefficients c[u,g] = w[s0+g] / (wsum_u + eps) (zeroed outside the
          window / for padding slots), plus the cheap packed outputs
          (idx, conf, slot_mask).
  device: indirect-DMA gather of the 5-row contiguous window per boundary slot
          (one 20KB descriptor per slot), DVE weighted combine, PE transpose,
          fp32r matmul against resident W^T, DMA out.
  host:   slice to U slots, add bias, apply slot mask.
"""

import os
import numpy as np

B, L, D = 8, 4096, 1024
POOL = 5
EPS = 1e-6
N_CORES = 8
P = 128
KC = D // P  # 8 contraction chunks

# Filled by the last kernel() call when BASS_KERNEL_PROFILE=1.
LAST_EXEC_NS = None
LAST_RESULTS = None

_nc_cache = {}


def _build_nc(m_tiles: int):
    import concourse.bass as bass
    import concourse.mybir as mybir
    import concourse.tile as tile
    from concourse import bacc
    from concourse.masks import make_identity

    f32 = mybir.dt.float32
    f32r = mybir.dt.float32r
    i32 = mybir.dt.int32

    nc = bacc.Bacc("TRN2", target_bir_lowering=False, debug=False, num_devices=N_CORES)
    x_d = nc.dram_tensor("x", [L, D], f32, kind="ExternalInput").ap()
    wt_d = nc.dram_tensor("wt", [D, D], f32r, kind="ExternalInput").ap()
    offs_d = nc.dram_tensor("offs", [P, m_tiles], i32, kind="ExternalInput").ap()
    coefs_d = nc.dram_tensor(
        "coefs", [P, POOL * m_tiles], f32, kind="ExternalInput"
    ).ap()
    out_d = nc.dram_tensor("out", [m_tiles * P, D], f32, kind="ExternalOutput").ap()

    with tile.TileContext(nc) as tc:
        with (
            tc.tile_pool(name="const", bufs=1) as cpool,
            tc.tile_pool(name="w", bufs=1) as wpool,
            tc.tile_pool(name="gather", bufs=3) as gpool,
            tc.tile_pool(name="acc", bufs=2) as apool,
            tc.tile_pool(name="lhst", bufs=2) as lpool,
            tc.tile_pool(name="outsb", bufs=2) as opool,
            tc.tile_pool(name="pst", bufs=2, space="PSUM") as pst_pool,
            tc.tile_pool(name="pso", bufs=2, space="PSUM") as pso_pool,
        ):
            ident = cpool.tile([P, P], f32, tag="ident")
            make_identity(nc, ident[:])
            offs_sb = cpool.tile([P, m_tiles], i32, tag="offs")
            nc.sync.dma_start(out=offs_sb[:], in_=offs_d)
            coefs_sb = cpool.tile([P, POOL * m_tiles], f32, tag="coefs")
            nc.sync.dma_start(out=coefs_sb[:], in_=coefs_d)

            w_sb = []
            for k in range(KC):
                wt_k = wpool.tile([P, D], f32r, tag=f"w{k}")
                nc.sync.dma_start(out=wt_k[:], in_=wt_d[k * P : (k + 1) * P, :])
                w_sb.append(wt_k)

            for mt in range(m_tiles):
                y = gpool.tile([P, POOL * D], f32, tag="y")
                nc.gpsimd.indirect_dma_start(
                    out=y[:],
                    out_offset=None,
                    in_=x_d,
                    in_offset=bass.IndirectOffsetOnAxis(
                        ap=offs_sb[:, mt : mt + 1], axis=0
                    ),
                )
                acc = apool.tile([P, D], f32, tag="acc")
                nc.any.tensor_scalar_mul(
                    acc[:], y[:, 0:D], coefs_sb[:, mt * POOL : mt * POOL + 1]
                )
                for g in range(1, POOL):
                    nc.vector.affine_then_add(
                        out=acc[:],
                        in0=y[:, g * D : (g + 1) * D],
                        in1=acc[:],
                        scale=coefs_sb[:, mt * POOL + g : mt * POOL + g + 1],
                        bias=0.0,
                    )

                lhsts = []
                for k in range(KC):
                    pst = pst_pool.tile([P, P], f32, tag="pst")
                    nc.tensor.transpose(
                        out=pst[:], in_=acc[:, k * P : (k + 1) * P], identity=ident[:]
                    )
                    lt = lpool.tile([P, P], f32r, tag=f"lt{k}")
                    nc.any.tensor_copy(lt[:], pst[:])
                    lhsts.append(lt)

                out_sb = opool.tile([P, D], f32, tag="osb")
                for n in range(2):
                    pso = pso_pool.tile([P, 512], f32, tag="pso")
                    for k in range(KC):
                        nc.tensor.matmul(
                            pso[:],
                            lhsts[k][:],
                            w_sb[k][:, n * 512 : (n + 1) * 512],
                            start=(k == 0),
                            stop=(k == KC - 1),
                        )
                    nc.any.tensor_copy(out_sb[:, n * 512 : (n + 1) * 512], pso[:])
                nc.sync.dma_start(
                    out=out_d[mt * P : (mt + 1) * P, :], in_=out_sb[:]
                )
    nc.compile()
    return nc


def _host_prep(boundary_mask, change_score, boundary_confidence):
    """Per-row boundary packing metadata. All O(B*L) scalar work."""
    mask = np.asarray(boundary_mask).astype(bool)
    w = np.asarray(change_score).astype(np.float64)
    conf_in = np.asarray(boundary_confidence).astype(np.float32)

    counts = mask.sum(axis=1).astype(np.int64)
    U = max(int(counts.max()), 1)
    m_tiles = max(1, -(-U // P))
    M = m_tiles * P

    offs = np.zeros((B, P, m_tiles), dtype=np.int32)
    coefs = np.zeros((B, P, POOL * m_tiles), dtype=np.float32)
    idx = np.zeros((B, U), dtype=np.int32)
    conf = np.zeros((B, U), dtype=np.float32)
    slot_mask = np.zeros((B, U), dtype=bool)

    g_off = np.arange(POOL, dtype=np.int64)
    for r in range(B):
        pos = np.nonzero(mask[r])[0]
        cnt = len(pos)
        if cnt == 0:
            continue
        idx[r, :cnt] = pos
        conf[r, :cnt] = conf_in[r, pos]
        slot_mask[r, :cnt] = True

        s0 = np.maximum(pos - (POOL - 1), 0)
        rows = s0[:, None] + g_off[None, :]  # (cnt, POOL)
        valid = rows <= pos[:, None]
        wrows = w[r, rows] * valid
        wsum = wrows.sum(axis=1)
        c = (wrows / (wsum + EPS)[:, None]).astype(np.float32)

        # slot u = mt*P + p  ->  offs[p, mt], coefs[p, mt*POOL+g]
        u = np.arange(cnt)
        pp, mm = u % P, u // P
        offs[r, pp, mm] = s0.astype(np.int32)
        for g in range(POOL):
            coefs[r, pp, mm * POOL + g] = c[:, g]

    return counts, U, m_tiles, M, offs, coefs, idx, conf, slot_mask


def _install_ntff_hook_shim():
    """Provide antenv.axon_hooks (absent in this image) so bass_utils can
    NTFF-profile under axon. Mirrors trn_agent_boot's ctypes hook."""
    import sys

    if "antenv.axon_hooks" in sys.modules:
        return
    import contextlib
    import ctypes
    import types

    so_path = "/opt/axon/libaxon_pjrt.so"
    lib = ctypes.CDLL(so_path)
    if not hasattr(lib, "axon_start_nrt_profile"):
        raise RuntimeError("libaxon_pjrt.so lacks axon_start_nrt_profile")
    lib.axon_start_nrt_profile.argtypes = [
        ctypes.POINTER(ctypes.c_int64),
        ctypes.c_size_t,
    ]
    lib.axon_start_nrt_profile.restype = ctypes.c_int64
    lib.axon_stop_nrt_profile.argtypes = [ctypes.c_char_p]
    lib.axon_stop_nrt_profile.restype = ctypes.c_int64

    @contextlib.contextmanager
    def _hook(output_dir, device_ids):
        import jax

        jax.devices()
        if device_ids:
            ids = (ctypes.c_int64 * len(device_ids))(*device_ids)
            rc = lib.axon_start_nrt_profile(ids, len(device_ids))
        else:
            rc = lib.axon_start_nrt_profile(None, 0)
        if rc != 0:
            raise RuntimeError(f"axon_start_nrt_profile rc={rc}")
        try:
            yield
        finally:
            n = lib.axon_stop_nrt_profile(str(output_dir).encode())
            print(f"ntff profile: {n} file(s) written to {output_dir}")

    mod = types.ModuleType("antenv.axon_hooks")
    mod.get_axon_ntff_profile_hook = lambda: _hook
    mod.set_axon_ntff_profile_hook = lambda h: None
    sys.modules["antenv.axon_hooks"] = mod


def kernel(frame_embeddings, boundary_mask, change_score, boundary_confidence, W, b):
    global LAST_EXEC_NS, LAST_RESULTS
    from concourse.bass_utils import run_bass_kernel_spmd

    x = np.ascontiguousarray(np.asarray(frame_embeddings), dtype=np.float32)
    W = np.asarray(W).astype(np.float32)
    b = np.asarray(b).astype(np.float32)

    counts, U, m_tiles, M, offs, coefs, idx, conf, slot_mask = _host_prep(
        boundary_mask, change_score, boundary_confidence
    )

    if m_tiles not in _nc_cache:
        _nc_cache[m_tiles] = _build_nc(m_tiles)
    nc = _nc_cache[m_tiles]

    wt = np.ascontiguousarray(W.T)
    in_maps = [
        {
            "x": x[r],
            "wt": wt,
            "offs": offs[r],
            "coefs": coefs[r],
        }
        for r in range(N_CORES)
    ]

    profile = os.environ.get("BASS_KERNEL_PROFILE", "0") == "1"
    if profile:
        try:
            _install_ntff_hook_shim()
        except Exception as e:
            print(f"ntff hook shim failed ({e}); running without profile")
            profile = False
    res = run_bass_kernel_spmd(
        nc, in_maps, list(range(N_CORES)), trace=profile
    )
    LAST_RESULTS = res
    LAST_EXEC_NS = res.exec_time_ns

    dev = np.stack([res.results[r]["out"] for r in range(N_CORES)], axis=0)
    chunks = dev[:, :U, :] + b[None, None, :]
    chunks = np.where(slot_mask[..., None], chunks, np.float32(0.0)).astype(np.float32)
    return chunks, slot_mask, idx, conf
